# revision 14
# baseline (speedup 1.0000x reference)
"""BitNet-style ReLU^2 FFN (bit_linear -> relu^2 -> bit_linear) on 8 TRN2 NeuronCores.

Strategy
--------
Data-parallel over tokens (8192 tokens -> 1024/core), zero collectives.

The forward pass of bit_linear is pure integer arithmetic after two
observations:
  * act_quant(rms_norm(x)): the rsqrt factor cancels inside the round:
        int_x = round(x * 127 / max|x|)           (values in [-127, 127])
    and the dequant scale is  max|x_n| / 127  per token.
  * weight_quant(W) is ternary {-1,0,1} times a per-tensor scalar.

int8-range integers and ternary values are exact in bf16, and the PE
accumulates in fp32 (exact up to 2^24 >> 8192*127), so both matmuls run
as bf16 PE matmuls that produce *exact* integer results; all scales are
applied at PSUM eviction. relu^2 also stays in the integer domain
(per-token positive scales cancel inside layer-2's quantization round).

Per-core pipeline:
  0a. per-token stats of x (abs-max, sum x^2)
  0b. quantize x -> bf16 ints, PE-transpose to [K, tok] layout
  1.  L1 matmul (W1^T streamed in 512-col chunks, all 1024 tokens resident),
      relu^2 on ACT, per-chunk stats (max, bn_stats), spill integer g to HBM
      (32 MB fp32/core; SBUF cannot hold tokens x 8192 fp32)
  1.5 finalize per-token layer-2 scales
  2a. reload g, quantize -> bf16 ints, PE-transpose to [K, tok]
  2b. L2 matmul (W2^T streamed), scale by beta[token] at eviction, store y

Rounding uses the fp32 magic-number trick (v*s + 2^23+2^22) - (2^23+2^22)
which is exact RNE round-to-integer on any IEEE fp32 ALU (matches jnp.round).
Weight quantization (per-tensor, data-dependent scalar) runs on host in jax
CPU fp32, matching the reference bit-for-bit.
"""

import os

os.environ.setdefault("JAX_PLATFORMS", "axon,cpu")

import numpy as np
import ml_dtypes
from contextlib import ExitStack

EPS = 1e-5
MAGIC = float(12582912.0)  # 2**23 + 2**22: fp32 RNE round-to-int magic
P = 128
D = 2048          # d_model
H = 8192          # hidden
N_CORES = 8
TOK = 8192 // N_CORES   # tokens per core = 1024
TT = TOK // P           # 8 token tiles per core
HN = 512                # L1 output chunk (one fp32 PSUM bank)
HC = H // HN            # 16 hid chunks
KB1 = D // P            # 16 contraction blocks for L1
KB2 = H // P            # 64 contraction blocks for L2
DN = 512                # L2 output chunk
DC = D // DN            # 4 d_model chunks

_CACHE = {}


def _build_nc():
    import concourse.bass as bass
    import concourse.tile as tile
    from concourse import bacc, mybir
    from concourse.masks import make_identity

    f32 = mybir.dt.float32
    bf16 = mybir.dt.bfloat16
    AF = mybir.ActivationFunctionType
    OP = mybir.AluOpType
    AX = mybir.AxisListType

    nc = bacc.Bacc(None, target_bir_lowering=False)

    x_ext = nc.declare_dram_parameter("x", [TOK, D], f32, isOutput=False)
    w1_ext = nc.declare_dram_parameter("w1t", [HC, P, KB1, HN], bf16, isOutput=False)
    w2_ext = nc.declare_dram_parameter("w2t", [DC, P, KB2, DN], bf16, isOutput=False)
    cst_ext = nc.declare_dram_parameter("cst", [1, 8], f32, isOutput=False)
    y_ext = nc.declare_dram_parameter("y", [TOK, D], f32, isOutput=True)

    z_dram = nc.dram_tensor("zspill", [TT, HC, P, HN], f32)
    gqT_drams = [nc.dram_tensor(f"gqTspill{tt}", [P, KB2, P], bf16)
                 for tt in range(TT)]

    with tile.TileContext(nc) as tc, ExitStack() as ctx:
        const = ctx.enter_context(tc.tile_pool(name="const", bufs=1))
        stats = ctx.enter_context(tc.tile_pool(name="stats", bufs=1))

        ident = const.tile([P, P], bf16)
        make_identity(nc, ident)
        cst = const.tile([P, 8], f32)
        nc.sync.dma_start(out=cst, in_=cst_ext[:].to_broadcast([P, 8]))
        eps_c = const.tile([P, 1], f32)
        nc.vector.memset(eps_c, EPS)
        nmag_c = const.tile([P, 1], f32)
        nc.vector.memset(nmag_c, -MAGIC)

        sgam = stats.tile([P, TT], f32)     # quant scale for integer g
        beta = stats.tile([P, TT], f32)     # final per-token output scale

        # ---------------- phase 0/1: layer 1 ----------------
        with tc.tile_pool(name="xtile", bufs=3) as xp, \
             tc.tile_pool(name="xtmp", bufs=2) as xtp, \
             tc.tile_pool(name="xq", bufs=2) as xqp, \
             tc.tile_pool(name="xT", bufs=1) as xTp, \
             tc.tile_pool(name="w1", bufs=2) as w1p, \
             tc.tile_pool(name="z1", bufs=2) as z1p, \
             tc.tile_pool(name="zq", bufs=3) as zqp, \
             tc.tile_pool(name="stats1", bufs=1) as st1p, \
             tc.tile_pool(name="ps1", bufs=3, space="PSUM") as ps1p, \
             tc.tile_pool(name="psT", bufs=2, space="PSUM") as psTp:

            m1 = st1p.tile([P, TT], f32)       # per-token abs-max of x
            ss1 = st1p.tile([P, TT], f32)      # per-token sum x^2
            alpha2 = st1p.tile([P, TT], f32)   # (L1 dequant scale)^2
            gmax_c = st1p.tile([P, TT, HC], f32)
            gst6 = st1p.tile([P, TT, HC, 6], f32)
            tmp_a = st1p.tile([P, TT], f32)
            tmp_b = st1p.tile([P, TT], f32)
            tmp_c = st1p.tile([P, TT], f32)
            mv = st1p.tile([P, TT, 2], f32)
            va = st1p.tile([P, TT], f32)
            nr1 = st1p.tile([P, TT], f32)
            nr2 = st1p.tile([P, TT], f32)
            msq = st1p.tile([P, TT], f32)
            a4 = st1p.tile([P, TT], f32)

            # --- 0: per-token-tile stats -> scales -> quantize -> transpose.
            # Per-tt scale chains (not batched) so the first L1 matmuls can
            # start as soon as token tile 0 is prepared. ---
            rs1 = st1p.tile([P, TT], f32)
            mxn = st1p.tile([P, TT], f32)
            sig1 = st1p.tile([P, TT], f32)
            xTs = []
            for tt in range(TT):
                sl = slice(tt, tt + 1)
                xt = xp.tile([P, D], f32)
                nc.sync.dma_start(out=xt, in_=x_ext[tt * P:(tt + 1) * P, :])
                nc.vector.tensor_reduce(
                    out=m1[:, sl], in_=xt, axis=AX.X, op=OP.max,
                    apply_absolute_value=True)
                xsq = xtp.tile([P, D], f32)
                nc.scalar.activation(
                    out=xsq, in_=xt, func=AF.Square,
                    accum_out=ss1[:, sl])
                # rs1 = 1/sqrt(mean(x^2)+eps), Newton-refined
                nc.vector.tensor_scalar(out=va[:, sl], in0=ss1[:, sl],
                                        scalar1=1.0 / D, scalar2=EPS,
                                        op0=OP.mult, op1=OP.add)
                nc.scalar.activation(out=rs1[:, sl], in_=va[:, sl], func=AF.Sqrt)
                nc.vector.reciprocal(out=rs1[:, sl], in_=rs1[:, sl])
                nc.vector.tensor_tensor(out=nr1[:, sl], in0=rs1[:, sl],
                                        in1=rs1[:, sl], op=OP.mult)
                nc.vector.tensor_tensor(out=nr1[:, sl], in0=nr1[:, sl],
                                        in1=va[:, sl], op=OP.mult)
                nc.vector.tensor_scalar(out=nr1[:, sl], in0=nr1[:, sl],
                                        scalar1=3.0, scalar2=-0.5,
                                        op0=OP.subtract, op1=OP.mult)
                nc.vector.tensor_tensor(out=rs1[:, sl], in0=rs1[:, sl],
                                        in1=nr1[:, sl], op=OP.mult)
                # max|xn| clipped
                nc.vector.tensor_tensor(out=mxn[:, sl], in0=m1[:, sl],
                                        in1=rs1[:, sl], op=OP.mult)
                nc.vector.tensor_scalar_max(out=mxn[:, sl], in0=mxn[:, sl],
                                            scalar1=EPS)
                # sig1 = rs1 * 127 / maxxn (Newton-refined reciprocal)
                nc.vector.reciprocal(out=nr1[:, sl], in_=mxn[:, sl])
                nc.vector.tensor_tensor(out=nr2[:, sl], in0=nr1[:, sl],
                                        in1=mxn[:, sl], op=OP.mult)
                nc.vector.tensor_scalar(out=nr2[:, sl], in0=nr2[:, sl],
                                        scalar1=2.0, scalar2=-1.0,
                                        op0=OP.subtract, op1=OP.mult)
                nc.vector.tensor_tensor(out=nr1[:, sl], in0=nr1[:, sl],
                                        in1=nr2[:, sl], op=OP.mult)
                nc.vector.tensor_tensor(out=sig1[:, sl], in0=rs1[:, sl],
                                        in1=nr1[:, sl], op=OP.mult)
                nc.vector.tensor_scalar_mul(out=sig1[:, sl], in0=sig1[:, sl],
                                            scalar1=127.0)
                # quantize + transpose this token tile
                t = xtp.tile([P, D], f32)
                nc.vector.tensor_scalar(out=t, in0=xt, scalar1=sig1[:, sl],
                                        scalar2=MAGIC, op0=OP.mult, op1=OP.add)
                xq = xqp.tile([P, D], bf16)
                nc.scalar.activation(out=xq, in_=t, func=AF.Identity, bias=nmag_c)
                xT = xTp.tile([P, KB1, P], bf16, tag=f"xT{tt}")
                xTs.append(xT)
                for g4 in range(KB1 // 4):
                    pt = psTp.tile([P, 4 * P], bf16)
                    for j in range(4):
                        kb = g4 * 4 + j
                        nc.tensor.transpose(
                            pt[:, j * P:(j + 1) * P],
                            xq[:, kb * P:(kb + 1) * P], ident)
                    nc.vector.tensor_copy(
                        out=xT[:, g4 * 4:(g4 + 1) * 4, :],
                        in_=pt.rearrange("p (j t) -> p j t", j=4))
            # alpha^2, batched
            nc.vector.tensor_scalar(out=tmp_c, in0=mxn, scalar1=cst[:, 0:1],
                                    scalar2=None, op0=OP.mult)
            nc.vector.tensor_tensor(out=alpha2, in0=tmp_c, in1=tmp_c, op=OP.mult)

            # --- 1: L1 matmuls + relu^2 + stats + spill ---
            for hc in range(HC):
                w1c = w1p.tile([P, KB1, HN], bf16)
                nc.sync.dma_start(out=w1c, in_=w1_ext[hc])
                for tt in range(TT):
                    ps = ps1p.tile([P, HN], f32)
                    for kb in range(KB1):
                        nc.tensor.matmul(
                            ps, lhsT=xTs[tt][:, kb, :],
                            rhs=w1c[:, kb, :],
                            start=(kb == 0), stop=(kb == KB1 - 1))
                    z1 = z1p.tile([P, HN], f32)
                    nc.scalar.activation(out=z1, in_=ps, func=AF.Relu)
                    zq = zqp.tile([P, HN], f32)
                    nc.scalar.activation(out=zq, in_=z1, func=AF.Square)
                    nc.vector.tensor_reduce(
                        out=gmax_c[:, tt, hc:hc + 1], in_=zq, axis=AX.X, op=OP.max)
                    nc.vector.bn_stats(out=gst6[:, tt, hc, :], in_=zq)
                    nc.gpsimd.dma_start(out=z_dram[tt, hc], in_=zq)

            # ---------- phase 1.5: layer-2 scales (inside phase-1 scope) ----------
            nc.vector.tensor_reduce(out=tmp_a, in_=gmax_c, axis=AX.X, op=OP.max)
            for tt in range(TT):
                nc.vector.bn_aggr(out=mv[:, tt, :], in_=gst6[:, tt])
            # E[g_int^2] = var + mean^2
            nc.vector.tensor_tensor(out=msq, in0=mv[:, :, 0], in1=mv[:, :, 0], op=OP.mult)
            nc.vector.tensor_tensor(out=msq, in0=msq, in1=mv[:, :, 1], op=OP.add)
            # true-scale: g_true = g_int * alpha^2
            nc.vector.tensor_tensor(out=a4, in0=alpha2, in1=alpha2, op=OP.mult)
            nc.vector.tensor_tensor(out=msq, in0=msq, in1=a4, op=OP.mult)  # E[g_true^2]
            # rsd = rsqrt(E + eps), Newton-refined
            nc.vector.tensor_scalar(out=va, in0=msq, scalar1=EPS,
                                    scalar2=None, op0=OP.add)
            nc.scalar.activation(out=msq, in_=va, func=AF.Sqrt)
            nc.vector.reciprocal(out=msq, in_=msq)
            nc.vector.tensor_tensor(out=nr1, in0=msq, in1=msq, op=OP.mult)
            nc.vector.tensor_tensor(out=nr1, in0=nr1, in1=va, op=OP.mult)
            nc.vector.tensor_scalar(out=nr1, in0=nr1, scalar1=3.0, scalar2=-0.5,
                                    op0=OP.subtract, op1=OP.mult)
            nc.vector.tensor_tensor(out=msq, in0=msq, in1=nr1, op=OP.mult)  # rsd
            # max|g_n| = gmax_int * alpha^2 * rsd, clipped
            nc.vector.tensor_tensor(out=tmp_b, in0=tmp_a, in1=alpha2, op=OP.mult)
            nc.vector.tensor_tensor(out=tmp_b, in0=tmp_b, in1=msq, op=OP.mult)
            nc.vector.tensor_scalar_max(out=tmp_b, in0=tmp_b, scalar1=EPS)
            # beta = maxgn_c * meanabsW2/127  (c2 = cst[1])
            nc.vector.tensor_scalar(out=beta, in0=tmp_b, scalar1=cst[:, 1:2],
                                    scalar2=None, op0=OP.mult)
            # sgam = alpha^2 * rsd * 127 / maxgn_c  (Newton-refined reciprocal)
            nc.vector.reciprocal(out=nr1, in_=tmp_b)
            nc.vector.tensor_tensor(out=nr2, in0=nr1, in1=tmp_b, op=OP.mult)
            nc.vector.tensor_scalar(out=nr2, in0=nr2, scalar1=2.0, scalar2=-1.0,
                                    op0=OP.subtract, op1=OP.mult)
            nc.vector.tensor_tensor(out=nr1, in0=nr1, in1=nr2, op=OP.mult)
            nc.vector.tensor_tensor(out=sgam, in0=alpha2, in1=msq, op=OP.mult)
            nc.vector.tensor_tensor(out=sgam, in0=sgam, in1=nr1, op=OP.mult)
            nc.vector.tensor_scalar_mul(out=sgam, in0=sgam, scalar1=127.0)

        # ---------------- phase 2: layer 2 ----------------
        with tc.tile_pool(name="zl", bufs=3) as zlp, \
             tc.tile_pool(name="t2", bufs=2) as t2p, \
             tc.tile_pool(name="gq", bufs=2) as gqp, \
             tc.tile_pool(name="gst", bufs=3) as gstp, \
             tc.tile_pool(name="gTt", bufs=3) as gTp, \
             tc.tile_pool(name="w2", bufs=2) as w2p, \
             tc.tile_pool(name="ysb", bufs=2) as yp, \
             tc.tile_pool(name="ps2", bufs=3, space="PSUM") as ps2p, \
             tc.tile_pool(name="psT2", bufs=2, space="PSUM") as psT2p:

            # --- 2a: reload integer g, quantize, PE-transpose, spill to DRAM
            # in a partition-contiguous [P, KB2, P] layout per token tile ---
            for tt in range(TT):
                for hc in range(HC):
                    zl = zlp.tile([P, HN], f32)
                    nc.gpsimd.dma_start(out=zl, in_=z_dram[tt, hc])
                    t2 = t2p.tile([P, HN], f32)
                    nc.vector.tensor_scalar(out=t2, in0=zl,
                                            scalar1=sgam[:, tt:tt + 1],
                                            scalar2=MAGIC, op0=OP.mult, op1=OP.add)
                    gq = gqp.tile([P, HN], bf16)
                    nc.scalar.activation(out=gq, in_=t2, func=AF.Identity, bias=nmag_c)
                    pt = psT2p.tile([P, 4 * P], bf16)
                    for j in range(4):
                        nc.tensor.transpose(
                            pt[:, j * P:(j + 1) * P],
                            gq[:, j * P:(j + 1) * P], ident)
                    gst = gstp.tile([P, 4, P], bf16)
                    nc.vector.tensor_copy(
                        out=gst, in_=pt.rearrange("p (j t) -> p j t", j=4))
                    nc.gpsimd.dma_start(
                        out=gqT_drams[tt][:, hc * 4:(hc + 1) * 4, :], in_=gst)

            # --- 2b: L2 matmuls, N=512 streams; gT tiles stream back via
            # plain contiguous DMA (16 KB per partition line) ---
            for dc in range(DC):
                w2c = w2p.tile([P, KB2, DN], bf16)
                nc.gpsimd.dma_start(out=w2c, in_=w2_ext[dc])
                for tt in range(TT):
                    gTt = gTp.tile([P, KB2, P], bf16)
                    nc.sync.dma_start(out=gTt, in_=gqT_drams[tt][:])
                    ps = ps2p.tile([P, DN], f32)
                    for kb in range(KB2):
                        nc.tensor.matmul(
                            ps, lhsT=gTt[:, kb, :],
                            rhs=w2c[:, kb, :],
                            start=(kb == 0), stop=(kb == KB2 - 1))
                    ysb = yp.tile([P, DN], f32)
                    nc.scalar.activation(out=ysb, in_=ps, func=AF.Copy,
                                         scale=beta[:, tt:tt + 1])
                    nc.scalar.dma_start(
                        out=y_ext[tt * P:(tt + 1) * P, dc * DN:(dc + 1) * DN],
                        in_=ysb)

    nc.compile()
    return nc


def _host_prep(W1, W2):
    """Quantize weights exactly as the jax fp32 reference does (on CPU)."""
    import jax
    import jax.numpy as jnp

    cpu = jax.devices("cpu")[0]
    out = {}
    for name, W in (("w1", W1), ("w2", W2)):
        with jax.default_device(cpu):
            Wj = jnp.asarray(W)
            scale = 1.0 / jnp.clip(jnp.mean(jnp.abs(Wj)), EPS)
            Wq = jnp.clip(jnp.round(Wj * scale), -1.0, 1.0)
            out[name + "_int"] = np.asarray(Wq).astype(np.float32)
            # effective per-tensor weight magnitude, as the fp32 reference divides
            out[name + "_q"] = float(np.float32(1.0) / np.float32(scale))
    return out


def kernel(x, W1, W2):
    from concourse.bass_utils import run_bass_kernel_spmd

    if "nc" not in _CACHE:
        _CACHE["nc"] = _build_nc()
    nc = _CACHE["nc"]

    xf = np.ascontiguousarray(x.reshape(-1, D).astype(np.float32, copy=False))
    wp = _host_prep(np.asarray(W1, np.float32), np.asarray(W2, np.float32))

    bf16 = ml_dtypes.bfloat16
    # W1int [H, D] -> W1^T [D, H] -> [hc, p, kb, hn]
    w1t = np.ascontiguousarray(
        wp["w1_int"].T.reshape(KB1, P, HC, HN).transpose(2, 1, 0, 3)
    ).astype(bf16)
    # W2int [D, H] -> W2^T [H, D] -> [dc, p, kb, dn]
    w2t = np.ascontiguousarray(
        wp["w2_int"].T.reshape(KB2, P, DC, DN).transpose(2, 1, 0, 3)
    ).astype(bf16)

    cst = np.zeros((1, 8), np.float32)
    cst[0, 0] = np.float32(wp["w1_q"]) / np.float32(127.0)
    cst[0, 1] = np.float32(wp["w2_q"]) / np.float32(127.0)

    in_maps = []
    for c in range(N_CORES):
        in_maps.append({
            "x": xf[c * TOK:(c + 1) * TOK],
            "w1t": w1t,
            "w2t": w2t,
            "cst": cst,
        })

    res = run_bass_kernel_spmd(nc, in_maps, list(range(N_CORES)),
                               trace=_CACHE.get("trace", False))
    _CACHE["last_res"] = res
    y = np.concatenate([res.results[c]["y"] for c in range(N_CORES)], axis=0)
    return y.reshape(x.shape[0], x.shape[1], D).astype(np.float32, copy=False)


# revision 15
# speedup vs baseline: 1.1427x; 1.1427x over previous
"""BitNet-style ReLU^2 FFN (bit_linear -> relu^2 -> bit_linear) on 8 TRN2 NeuronCores.

Strategy
--------
Data-parallel over tokens (8192 tokens -> 1024/core), zero collectives.

The forward pass of bit_linear is pure integer arithmetic after two
observations:
  * act_quant(rms_norm(x)): the rsqrt factor cancels inside the round:
        int_x = round(x * 127 / max|x|)           (values in [-127, 127])
    and the dequant scale is  max|x_n| / 127  per token.
  * weight_quant(W) is ternary {-1,0,1} times a per-tensor scalar.

int8-range integers and ternary values are exact in bf16, and the PE
accumulates in fp32 (exact up to 2^24 >> 8192*127), so both matmuls run
as bf16 PE matmuls that produce *exact* integer results; all scales are
applied at PSUM eviction. relu^2 also stays in the integer domain
(per-token positive scales cancel inside layer-2's quantization round).

Per-core pipeline:
  0a. per-token stats of x (abs-max, sum x^2)
  0b. quantize x -> bf16 ints, PE-transpose to [K, tok] layout
  1.  L1 matmul (W1^T streamed in 512-col chunks, all 1024 tokens resident),
      relu^2 on ACT, per-chunk stats (max, bn_stats), spill integer g to HBM
      (32 MB fp32/core; SBUF cannot hold tokens x 8192 fp32)
  1.5 finalize per-token layer-2 scales
  2a. reload g, quantize -> bf16 ints, PE-transpose to [K, tok]
  2b. L2 matmul (W2^T streamed), scale by beta[token] at eviction, store y

Rounding uses the fp32 magic-number trick (v*s + 2^23+2^22) - (2^23+2^22)
which is exact RNE round-to-integer on any IEEE fp32 ALU (matches jnp.round).
Weight quantization (per-tensor, data-dependent scalar) runs on host in jax
CPU fp32, matching the reference bit-for-bit.
"""

import os

os.environ.setdefault("JAX_PLATFORMS", "axon,cpu")

import numpy as np
import ml_dtypes
from contextlib import ExitStack

EPS = 1e-5
MAGIC = float(12582912.0)  # 2**23 + 2**22: fp32 RNE round-to-int magic
P = 128
D = 2048          # d_model
H = 8192          # hidden
N_CORES = 8
TOK = 8192 // N_CORES   # tokens per core = 1024
TT = TOK // P           # 8 token tiles per core
HN = 512                # L1 output chunk (one fp32 PSUM bank)
HC = H // HN            # 16 hid chunks
KB1 = D // P            # 16 contraction blocks for L1
KB2 = H // P            # 64 contraction blocks for L2
DN = 512                # L2 output chunk
DC = D // DN            # 4 d_model chunks

_CACHE = {}


def _build_nc():
    import concourse.bass as bass
    import concourse.tile as tile
    from concourse import bacc, mybir
    from concourse.masks import make_identity

    f32 = mybir.dt.float32
    bf16 = mybir.dt.bfloat16
    AF = mybir.ActivationFunctionType
    OP = mybir.AluOpType
    AX = mybir.AxisListType

    nc = bacc.Bacc(None, target_bir_lowering=False)

    x_ext = nc.declare_dram_parameter("x", [TOK, D], f32, isOutput=False)
    w1_ext = nc.declare_dram_parameter("w1t", [HC, P, KB1, HN], bf16, isOutput=False)
    w2_ext = nc.declare_dram_parameter("w2t", [DC, P, KB2, DN], bf16, isOutput=False)
    cst_ext = nc.declare_dram_parameter("cst", [1, 8], f32, isOutput=False)
    y_ext = nc.declare_dram_parameter("y", [TOK, D], f32, isOutput=True)

    z_dram = nc.dram_tensor("zspill", [TT, HC, P, HN], f32)
    gqT_drams = [nc.dram_tensor(f"gqTspill{tt}", [P, KB2, P], bf16)
                 for tt in range(TT)]

    with tile.TileContext(nc) as tc, ExitStack() as ctx:
        const = ctx.enter_context(tc.tile_pool(name="const", bufs=1))
        stats = ctx.enter_context(tc.tile_pool(name="stats", bufs=1))

        ident = const.tile([P, P], bf16)
        make_identity(nc, ident)
        cst = const.tile([P, 8], f32)
        nc.sync.dma_start(out=cst, in_=cst_ext[:].to_broadcast([P, 8]))
        eps_c = const.tile([P, 1], f32)
        nc.vector.memset(eps_c, EPS)
        nmag_c = const.tile([P, 1], f32)
        nc.vector.memset(nmag_c, -MAGIC)

        sgam = stats.tile([P, TT], f32)     # quant scale for integer g
        beta = stats.tile([P, TT], f32)     # final per-token output scale

        # ---------------- phase 0/1: layer 1 ----------------
        with tc.tile_pool(name="xtile", bufs=3) as xp, \
             tc.tile_pool(name="xtmp", bufs=2) as xtp, \
             tc.tile_pool(name="xq", bufs=2) as xqp, \
             tc.tile_pool(name="xT", bufs=1) as xTp, \
             tc.tile_pool(name="w1", bufs=2) as w1p, \
             tc.tile_pool(name="z1", bufs=2) as z1p, \
             tc.tile_pool(name="zq", bufs=3) as zqp, \
             tc.tile_pool(name="stats1", bufs=1) as st1p, \
             tc.tile_pool(name="ps1", bufs=3, space="PSUM") as ps1p, \
             tc.tile_pool(name="psT", bufs=2, space="PSUM") as psTp:

            m1 = st1p.tile([P, TT], f32)       # per-token abs-max of x
            ss1 = st1p.tile([P, TT], f32)      # per-token sum x^2
            alpha2 = st1p.tile([P, TT], f32)   # (L1 dequant scale)^2
            gmax_c = st1p.tile([P, TT, HC], f32)
            gst6 = st1p.tile([P, TT, HC, 6], f32)
            tmp_a = st1p.tile([P, TT], f32)
            tmp_b = st1p.tile([P, TT], f32)
            tmp_c = st1p.tile([P, TT], f32)
            mv = st1p.tile([P, TT, 2], f32)
            va = st1p.tile([P, TT], f32)
            nr1 = st1p.tile([P, TT], f32)
            nr2 = st1p.tile([P, TT], f32)
            msq = st1p.tile([P, TT], f32)
            a4 = st1p.tile([P, TT], f32)

            # --- 0: per-token-tile stats -> scales -> quantize -> transpose.
            # Per-tt scale chains (not batched) so the first L1 matmuls can
            # start as soon as token tile 0 is prepared. ---
            rs1 = st1p.tile([P, TT], f32)
            mxn = st1p.tile([P, TT], f32)
            sig1 = st1p.tile([P, TT], f32)
            xTs = []
            for tt in range(TT):
                sl = slice(tt, tt + 1)
                xt = xp.tile([P, D], f32)
                nc.sync.dma_start(out=xt, in_=x_ext[tt * P:(tt + 1) * P, :])
                nc.vector.tensor_reduce(
                    out=m1[:, sl], in_=xt, axis=AX.X, op=OP.max,
                    apply_absolute_value=True)
                xsq = xtp.tile([P, D], f32)
                nc.scalar.activation(
                    out=xsq, in_=xt, func=AF.Square,
                    accum_out=ss1[:, sl])
                # rs1 = 1/sqrt(mean(x^2)+eps), Newton-refined
                nc.vector.tensor_scalar(out=va[:, sl], in0=ss1[:, sl],
                                        scalar1=1.0 / D, scalar2=EPS,
                                        op0=OP.mult, op1=OP.add)
                nc.scalar.activation(out=rs1[:, sl], in_=va[:, sl], func=AF.Sqrt)
                nc.vector.reciprocal(out=rs1[:, sl], in_=rs1[:, sl])
                nc.vector.tensor_tensor(out=nr1[:, sl], in0=rs1[:, sl],
                                        in1=rs1[:, sl], op=OP.mult)
                nc.vector.tensor_tensor(out=nr1[:, sl], in0=nr1[:, sl],
                                        in1=va[:, sl], op=OP.mult)
                nc.vector.tensor_scalar(out=nr1[:, sl], in0=nr1[:, sl],
                                        scalar1=3.0, scalar2=-0.5,
                                        op0=OP.subtract, op1=OP.mult)
                nc.vector.tensor_tensor(out=rs1[:, sl], in0=rs1[:, sl],
                                        in1=nr1[:, sl], op=OP.mult)
                # max|xn| clipped
                nc.vector.tensor_tensor(out=mxn[:, sl], in0=m1[:, sl],
                                        in1=rs1[:, sl], op=OP.mult)
                nc.vector.tensor_scalar_max(out=mxn[:, sl], in0=mxn[:, sl],
                                            scalar1=EPS)
                # sig1 = rs1 * 127 / maxxn (Newton-refined reciprocal)
                nc.vector.reciprocal(out=nr1[:, sl], in_=mxn[:, sl])
                nc.vector.tensor_tensor(out=nr2[:, sl], in0=nr1[:, sl],
                                        in1=mxn[:, sl], op=OP.mult)
                nc.vector.tensor_scalar(out=nr2[:, sl], in0=nr2[:, sl],
                                        scalar1=2.0, scalar2=-1.0,
                                        op0=OP.subtract, op1=OP.mult)
                nc.vector.tensor_tensor(out=nr1[:, sl], in0=nr1[:, sl],
                                        in1=nr2[:, sl], op=OP.mult)
                nc.vector.tensor_tensor(out=sig1[:, sl], in0=rs1[:, sl],
                                        in1=nr1[:, sl], op=OP.mult)
                nc.vector.tensor_scalar_mul(out=sig1[:, sl], in0=sig1[:, sl],
                                            scalar1=127.0)
                # quantize + transpose this token tile
                t = xtp.tile([P, D], f32)
                nc.vector.tensor_scalar(out=t, in0=xt, scalar1=sig1[:, sl],
                                        scalar2=MAGIC, op0=OP.mult, op1=OP.add)
                xq = xqp.tile([P, D], bf16)
                nc.scalar.activation(out=xq, in_=t, func=AF.Identity, bias=nmag_c)
                xT = xTp.tile([P, KB1, P], bf16, tag=f"xT{tt}")
                xTs.append(xT)
                for g4 in range(KB1 // 4):
                    pt = psTp.tile([P, 4 * P], bf16)
                    for j in range(4):
                        kb = g4 * 4 + j
                        nc.tensor.transpose(
                            pt[:, j * P:(j + 1) * P],
                            xq[:, kb * P:(kb + 1) * P], ident)
                    nc.vector.tensor_copy(
                        out=xT[:, g4 * 4:(g4 + 1) * 4, :],
                        in_=pt.rearrange("p (j t) -> p j t", j=4))
            # alpha^2, batched
            nc.vector.tensor_scalar(out=tmp_c, in0=mxn, scalar1=cst[:, 0:1],
                                    scalar2=None, op0=OP.mult)
            nc.vector.tensor_tensor(out=alpha2, in0=tmp_c, in1=tmp_c, op=OP.mult)

            # --- 1: L1 matmuls + relu^2 + stats + spill ---
            for hc in range(HC):
                w1c = w1p.tile([P, KB1, HN], bf16)
                nc.sync.dma_start(out=w1c, in_=w1_ext[hc])
                for tt in range(TT):
                    ps = ps1p.tile([P, HN], f32)
                    for kb in range(KB1):
                        nc.tensor.matmul(
                            ps, lhsT=xTs[tt][:, kb, :],
                            rhs=w1c[:, kb, :],
                            start=(kb == 0), stop=(kb == KB1 - 1))
                    z1 = z1p.tile([P, HN], f32)
                    nc.scalar.activation(out=z1, in_=ps, func=AF.Relu)
                    zq = zqp.tile([P, HN], f32)
                    nc.scalar.activation(out=zq, in_=z1, func=AF.Square)
                    nc.vector.tensor_reduce(
                        out=gmax_c[:, tt, hc:hc + 1], in_=zq, axis=AX.X, op=OP.max)
                    nc.vector.bn_stats(out=gst6[:, tt, hc, :], in_=zq)
                    nc.gpsimd.dma_start(out=z_dram[tt, hc], in_=zq)

            # ---------- phase 1.5: layer-2 scales (inside phase-1 scope) ----------
            nc.vector.tensor_reduce(out=tmp_a, in_=gmax_c, axis=AX.X, op=OP.max)
            for tt in range(TT):
                nc.vector.bn_aggr(out=mv[:, tt, :], in_=gst6[:, tt])
            # E[g_int^2] = var + mean^2
            nc.vector.tensor_tensor(out=msq, in0=mv[:, :, 0], in1=mv[:, :, 0], op=OP.mult)
            nc.vector.tensor_tensor(out=msq, in0=msq, in1=mv[:, :, 1], op=OP.add)
            # true-scale: g_true = g_int * alpha^2
            nc.vector.tensor_tensor(out=a4, in0=alpha2, in1=alpha2, op=OP.mult)
            nc.vector.tensor_tensor(out=msq, in0=msq, in1=a4, op=OP.mult)  # E[g_true^2]
            # rsd = rsqrt(E + eps), Newton-refined
            nc.vector.tensor_scalar(out=va, in0=msq, scalar1=EPS,
                                    scalar2=None, op0=OP.add)
            nc.scalar.activation(out=msq, in_=va, func=AF.Sqrt)
            nc.vector.reciprocal(out=msq, in_=msq)
            nc.vector.tensor_tensor(out=nr1, in0=msq, in1=msq, op=OP.mult)
            nc.vector.tensor_tensor(out=nr1, in0=nr1, in1=va, op=OP.mult)
            nc.vector.tensor_scalar(out=nr1, in0=nr1, scalar1=3.0, scalar2=-0.5,
                                    op0=OP.subtract, op1=OP.mult)
            nc.vector.tensor_tensor(out=msq, in0=msq, in1=nr1, op=OP.mult)  # rsd
            # max|g_n| = gmax_int * alpha^2 * rsd, clipped
            nc.vector.tensor_tensor(out=tmp_b, in0=tmp_a, in1=alpha2, op=OP.mult)
            nc.vector.tensor_tensor(out=tmp_b, in0=tmp_b, in1=msq, op=OP.mult)
            nc.vector.tensor_scalar_max(out=tmp_b, in0=tmp_b, scalar1=EPS)
            # beta = maxgn_c * meanabsW2/127  (c2 = cst[1])
            nc.vector.tensor_scalar(out=beta, in0=tmp_b, scalar1=cst[:, 1:2],
                                    scalar2=None, op0=OP.mult)
            # sgam = alpha^2 * rsd * 127 / maxgn_c  (Newton-refined reciprocal)
            nc.vector.reciprocal(out=nr1, in_=tmp_b)
            nc.vector.tensor_tensor(out=nr2, in0=nr1, in1=tmp_b, op=OP.mult)
            nc.vector.tensor_scalar(out=nr2, in0=nr2, scalar1=2.0, scalar2=-1.0,
                                    op0=OP.subtract, op1=OP.mult)
            nc.vector.tensor_tensor(out=nr1, in0=nr1, in1=nr2, op=OP.mult)
            nc.vector.tensor_tensor(out=sgam, in0=alpha2, in1=msq, op=OP.mult)
            nc.vector.tensor_tensor(out=sgam, in0=sgam, in1=nr1, op=OP.mult)
            nc.vector.tensor_scalar_mul(out=sgam, in0=sgam, scalar1=127.0)

        # ---------------- phase 2: layer 2 ----------------
        with tc.tile_pool(name="zl", bufs=3) as zlp, \
             tc.tile_pool(name="t2", bufs=2) as t2p, \
             tc.tile_pool(name="gq", bufs=2) as gqp, \
             tc.tile_pool(name="gst", bufs=3) as gstp, \
             tc.tile_pool(name="gTt", bufs=3) as gTp, \
             tc.tile_pool(name="w2", bufs=2) as w2p, \
             tc.tile_pool(name="ysb", bufs=2) as yp, \
             tc.tile_pool(name="ps2", bufs=3, space="PSUM") as ps2p, \
             tc.tile_pool(name="psT2", bufs=2, space="PSUM") as psT2p:

            # --- 2a: reload integer g, quantize, PE-transpose, spill to DRAM
            # in a partition-contiguous [P, KB2, P] layout per token tile ---
            for tt in range(TT):
                for hc in range(HC):
                    zl = zlp.tile([P, HN], f32)
                    nc.gpsimd.dma_start(out=zl, in_=z_dram[tt, hc])
                    t2 = t2p.tile([P, HN], f32)
                    nc.vector.tensor_scalar(out=t2, in0=zl,
                                            scalar1=sgam[:, tt:tt + 1],
                                            scalar2=MAGIC, op0=OP.mult, op1=OP.add)
                    gq = gqp.tile([P, HN], bf16)
                    nc.scalar.activation(out=gq, in_=t2, func=AF.Identity, bias=nmag_c)
                    pt = psT2p.tile([P, 4 * P], bf16)
                    for j in range(4):
                        nc.tensor.transpose(
                            pt[:, j * P:(j + 1) * P],
                            gq[:, j * P:(j + 1) * P], ident)
                    gst = gstp.tile([P, 4, P], bf16)
                    nc.vector.tensor_copy(
                        out=gst, in_=pt.rearrange("p (j t) -> p j t", j=4))
                    nc.gpsimd.dma_start(
                        out=gqT_drams[tt][:, hc * 4:(hc + 1) * 4, :], in_=gst)

            # --- 2b: L2 matmuls, N=512 streams; gT tiles stream back via
            # plain contiguous DMA (16 KB per partition line) ---
            for dc in range(DC):
                w2c = w2p.tile([P, KB2, DN], bf16)
                nc.scalar.dma_start(out=w2c, in_=w2_ext[dc])
                for tt in range(TT):
                    gTt = gTp.tile([P, KB2, P], bf16)
                    nc.sync.dma_start(out=gTt, in_=gqT_drams[tt][:])
                    ps = ps2p.tile([P, DN], f32)
                    for kb in range(KB2):
                        nc.tensor.matmul(
                            ps, lhsT=gTt[:, kb, :],
                            rhs=w2c[:, kb, :],
                            start=(kb == 0), stop=(kb == KB2 - 1))
                    ysb = yp.tile([P, DN], f32)
                    nc.scalar.activation(out=ysb, in_=ps, func=AF.Copy,
                                         scale=beta[:, tt:tt + 1])
                    nc.scalar.dma_start(
                        out=y_ext[tt * P:(tt + 1) * P, dc * DN:(dc + 1) * DN],
                        in_=ysb)

    nc.compile()
    return nc


def _host_prep(W1, W2):
    """Quantize weights exactly as the jax fp32 reference does (on CPU)."""
    import jax
    import jax.numpy as jnp

    cpu = jax.devices("cpu")[0]
    out = {}
    for name, W in (("w1", W1), ("w2", W2)):
        with jax.default_device(cpu):
            Wj = jnp.asarray(W)
            scale = 1.0 / jnp.clip(jnp.mean(jnp.abs(Wj)), EPS)
            Wq = jnp.clip(jnp.round(Wj * scale), -1.0, 1.0)
            out[name + "_int"] = np.asarray(Wq).astype(np.float32)
            # effective per-tensor weight magnitude, as the fp32 reference divides
            out[name + "_q"] = float(np.float32(1.0) / np.float32(scale))
    return out


def kernel(x, W1, W2):
    from concourse.bass_utils import run_bass_kernel_spmd

    if "nc" not in _CACHE:
        _CACHE["nc"] = _build_nc()
    nc = _CACHE["nc"]

    xf = np.ascontiguousarray(x.reshape(-1, D).astype(np.float32, copy=False))
    wp = _host_prep(np.asarray(W1, np.float32), np.asarray(W2, np.float32))

    bf16 = ml_dtypes.bfloat16
    # W1int [H, D] -> W1^T [D, H] -> [hc, p, kb, hn]
    w1t = np.ascontiguousarray(
        wp["w1_int"].T.reshape(KB1, P, HC, HN).transpose(2, 1, 0, 3)
    ).astype(bf16)
    # W2int [D, H] -> W2^T [H, D] -> [dc, p, kb, dn]
    w2t = np.ascontiguousarray(
        wp["w2_int"].T.reshape(KB2, P, DC, DN).transpose(2, 1, 0, 3)
    ).astype(bf16)

    cst = np.zeros((1, 8), np.float32)
    cst[0, 0] = np.float32(wp["w1_q"]) / np.float32(127.0)
    cst[0, 1] = np.float32(wp["w2_q"]) / np.float32(127.0)

    in_maps = []
    for c in range(N_CORES):
        in_maps.append({
            "x": xf[c * TOK:(c + 1) * TOK],
            "w1t": w1t,
            "w2t": w2t,
            "cst": cst,
        })

    res = run_bass_kernel_spmd(nc, in_maps, list(range(N_CORES)),
                               trace=_CACHE.get("trace", False))
    _CACHE["last_res"] = res
    y = np.concatenate([res.results[c]["y"] for c in range(N_CORES)], axis=0)
    return y.reshape(x.shape[0], x.shape[1], D).astype(np.float32, copy=False)


# revision 27
# speedup vs baseline: 1.1636x; 1.0184x over previous
"""BitNet-style ReLU^2 FFN (bit_linear -> relu^2 -> bit_linear) on 8 TRN2 NeuronCores.

Strategy
--------
Data-parallel over tokens (8192 tokens -> 1024/core), zero collectives.

The forward pass of bit_linear is pure integer arithmetic after two
observations:
  * act_quant(rms_norm(x)): the rsqrt factor cancels inside the round:
        int_x = round(x * 127 / max|x|)           (values in [-127, 127])
    and the dequant scale is  max|x_n| / 127  per token.
  * weight_quant(W) is ternary {-1,0,1} times a per-tensor scalar.

int8-range integers and ternary values are exact in bf16, and the PE
accumulates in fp32 (exact up to 2^24 >> 8192*127), so both matmuls run
as bf16 PE matmuls that produce *exact* integer results; all scales are
applied at PSUM eviction. relu^2 also stays in the integer domain
(per-token positive scales cancel inside layer-2's quantization round).

Per-core pipeline:
  0a. per-token stats of x (abs-max, sum x^2)
  0b. quantize x -> bf16 ints, PE-transpose to [K, tok] layout
  1.  L1 matmul (W1^T streamed in 512-col chunks, all 1024 tokens resident),
      relu^2 on ACT, per-chunk stats (max, bn_stats), spill integer g to HBM
      (32 MB fp32/core; SBUF cannot hold tokens x 8192 fp32)
  1.5 finalize per-token layer-2 scales
  2a. reload g, quantize -> bf16 ints, PE-transpose to [K, tok]
  2b. L2 matmul (W2^T streamed), scale by beta[token] at eviction, store y

Rounding uses the fp32 magic-number trick (v*s + 2^23+2^22) - (2^23+2^22)
which is exact RNE round-to-integer on any IEEE fp32 ALU (matches jnp.round).
Weight quantization (per-tensor, data-dependent scalar) runs on host in jax
CPU fp32, matching the reference bit-for-bit.
"""

import os

os.environ.setdefault("JAX_PLATFORMS", "axon,cpu")

import numpy as np
import ml_dtypes
from contextlib import ExitStack

EPS = 1e-5
MAGIC = float(12582912.0)   # 2**23 + 2**22: fp32 RNE round-to-int magic
MAGIC16 = 1536.0            # 2**10 + 2**9: same trick via the fp16 cast for
                            # values in [0, 512): ulp(fp16 @ 1536) == 1
P = 128
D = 2048          # d_model
H = 8192          # hidden
N_CORES = 8
TOK = 8192 // N_CORES   # tokens per core = 1024
TT = TOK // P           # 8 token tiles per core
HN = 512                # L1 output chunk (one fp32 PSUM bank)
HC = H // HN            # 16 hid chunks
KB1 = D // P            # 16 contraction blocks for L1
KB2 = H // P            # 64 contraction blocks for L2
DN = 512                # L2 output chunk
DC = D // DN            # 4 d_model chunks

_CACHE = {}


def _build_nc():
    import concourse.bass as bass
    import concourse.tile as tile
    from concourse import bacc, mybir
    from concourse.masks import make_identity

    f32 = mybir.dt.float32
    bf16 = mybir.dt.bfloat16
    fp16 = mybir.dt.float16
    AF = mybir.ActivationFunctionType
    OP = mybir.AluOpType
    AX = mybir.AxisListType

    nc = bacc.Bacc(None, target_bir_lowering=False)

    x_ext = nc.declare_dram_parameter("x", [TOK, D], f32, isOutput=False)
    w1_ext = nc.declare_dram_parameter("w1t", [HC, P, KB1, HN], bf16, isOutput=False)
    w2_ext = nc.declare_dram_parameter("w2t", [DC, P, KB2, DN], bf16, isOutput=False)
    cst_ext = nc.declare_dram_parameter("cst", [1, 8], f32, isOutput=False)
    y_ext = nc.declare_dram_parameter("y", [TOK, D], f32, isOutput=True)

    z_drams = [nc.dram_tensor(f"zspill{tt}", [HC, P, HN], f32)
               for tt in range(TT)]
    gqT_drams = [nc.dram_tensor(f"gqTspill{tt}", [P, KB2, P], bf16)
                 for tt in range(TT)]

    with tile.TileContext(nc) as tc, ExitStack() as ctx:
        const = ctx.enter_context(tc.tile_pool(name="const", bufs=1))
        stats = ctx.enter_context(tc.tile_pool(name="stats", bufs=1))

        ident = const.tile([P, P], bf16)
        make_identity(nc, ident)
        cst = const.tile([P, 8], f32)
        nc.sync.dma_start(out=cst, in_=cst_ext[:].to_broadcast([P, 8]))
        eps_c = const.tile([P, 1], f32)
        nc.vector.memset(eps_c, EPS)
        nmag_c = const.tile([P, 1], f32)
        nc.vector.memset(nmag_c, -MAGIC)

        # per-token-tile scalar scratch: columns are named in `C` below.
        # One tile per token tile keeps the dependency chains of different
        # token tiles fully decoupled in the scheduler.
        scs = [stats.tile([P, 24], f32, tag=f"sc{tt}") for tt in range(TT)]

        # ---------------- phase 0/1: layer 1 ----------------
        with tc.tile_pool(name="xtile", bufs=3) as xp, \
             tc.tile_pool(name="xtmp", bufs=2) as xtp, \
             tc.tile_pool(name="xq", bufs=2) as xqp, \
             tc.tile_pool(name="xT", bufs=1) as xTp, \
             tc.tile_pool(name="w1", bufs=2) as w1p, \
             tc.tile_pool(name="z1", bufs=2) as z1p, \
             tc.tile_pool(name="zq", bufs=3) as zqp, \
             tc.tile_pool(name="zsq", bufs=2) as zsqp, \
             tc.tile_pool(name="stats1", bufs=1) as st1p, \
             tc.tile_pool(name="ps1", bufs=3, space="PSUM") as ps1p, \
             tc.tile_pool(name="psT", bufs=2, space="PSUM") as psTp:

            gxs = [st1p.tile([P, HC], f32, tag=f"gx{tt}") for tt in range(TT)]
            g6s = [st1p.tile([P, HC, 6], f32, tag=f"g6{tt}") for tt in range(TT)]

            # named columns of the per-tt scalar tiles
            C_M1, C_SS, C_VA, C_RS, C_N1, C_N2, C_MXN, C_SIG, C_A2 = range(9)
            C_GMX, C_MV0, C_MV1, C_MSQ, C_A4, C_TB, C_BETA, C_SGAM = range(9, 17)

            def col(tt, c):
                return scs[tt][:, c:c + 1]

            # --- 0: per-token-tile stats -> scales -> quantize -> transpose.
            # Per-tt chains so the first L1 matmuls start as soon as token
            # tile 0 is prepared. ---
            xTs = []
            for tt in range(TT):
                xt = xp.tile([P, D], f32)
                nc.sync.dma_start(out=xt, in_=x_ext[tt * P:(tt + 1) * P, :])
                nc.vector.tensor_reduce(
                    out=col(tt, C_M1), in_=xt, axis=AX.X, op=OP.max,
                    apply_absolute_value=True)
                xsq = xtp.tile([P, D], f32)
                nc.scalar.activation(
                    out=xsq, in_=xt, func=AF.Square,
                    accum_out=col(tt, C_SS))
                # rs1 = 1/sqrt(mean(x^2)+eps), Newton-refined
                nc.vector.tensor_scalar(out=col(tt, C_VA), in0=col(tt, C_SS),
                                        scalar1=1.0 / D, scalar2=EPS,
                                        op0=OP.mult, op1=OP.add)
                nc.scalar.activation(out=col(tt, C_RS), in_=col(tt, C_VA),
                                     func=AF.Sqrt)
                nc.vector.reciprocal(out=col(tt, C_RS), in_=col(tt, C_RS))
                nc.vector.tensor_tensor(out=col(tt, C_N1), in0=col(tt, C_RS),
                                        in1=col(tt, C_RS), op=OP.mult)
                nc.vector.tensor_tensor(out=col(tt, C_N1), in0=col(tt, C_N1),
                                        in1=col(tt, C_VA), op=OP.mult)
                nc.vector.tensor_scalar(out=col(tt, C_N1), in0=col(tt, C_N1),
                                        scalar1=3.0, scalar2=-0.5,
                                        op0=OP.subtract, op1=OP.mult)
                nc.vector.tensor_tensor(out=col(tt, C_RS), in0=col(tt, C_RS),
                                        in1=col(tt, C_N1), op=OP.mult)
                # max|xn| clipped
                nc.vector.tensor_tensor(out=col(tt, C_MXN), in0=col(tt, C_M1),
                                        in1=col(tt, C_RS), op=OP.mult)
                nc.vector.tensor_scalar_max(out=col(tt, C_MXN),
                                            in0=col(tt, C_MXN), scalar1=EPS)
                # sig1 = rs1 * 127 / maxxn (Newton-refined reciprocal)
                nc.vector.reciprocal(out=col(tt, C_N1), in_=col(tt, C_MXN))
                nc.vector.tensor_tensor(out=col(tt, C_N2), in0=col(tt, C_N1),
                                        in1=col(tt, C_MXN), op=OP.mult)
                nc.vector.tensor_scalar(out=col(tt, C_N2), in0=col(tt, C_N2),
                                        scalar1=2.0, scalar2=-1.0,
                                        op0=OP.subtract, op1=OP.mult)
                nc.vector.tensor_tensor(out=col(tt, C_N1), in0=col(tt, C_N1),
                                        in1=col(tt, C_N2), op=OP.mult)
                nc.vector.tensor_tensor(out=col(tt, C_SIG), in0=col(tt, C_RS),
                                        in1=col(tt, C_N1), op=OP.mult)
                nc.vector.tensor_scalar_mul(out=col(tt, C_SIG),
                                            in0=col(tt, C_SIG), scalar1=127.0)
                # alpha^2 for this token tile
                nc.vector.tensor_scalar(out=col(tt, C_A2), in0=col(tt, C_MXN),
                                        scalar1=cst[:, 0:1], scalar2=None,
                                        op0=OP.mult)
                nc.vector.tensor_tensor(out=col(tt, C_A2), in0=col(tt, C_A2),
                                        in1=col(tt, C_A2), op=OP.mult)
                # quantize + transpose this token tile
                t = xtp.tile([P, D], f32)
                nc.vector.tensor_scalar(out=t, in0=xt, scalar1=col(tt, C_SIG),
                                        scalar2=MAGIC, op0=OP.mult, op1=OP.add)
                xq = xqp.tile([P, D], bf16)
                nc.scalar.activation(out=xq, in_=t, func=AF.Identity, bias=nmag_c)
                xT = xTp.tile([P, KB1, P], bf16, tag=f"xT{tt}")
                xTs.append(xT)
                for g4 in range(KB1 // 4):
                    pt = psTp.tile([P, 4 * P], bf16)
                    for j in range(4):
                        kb = g4 * 4 + j
                        nc.tensor.transpose(
                            pt[:, j * P:(j + 1) * P],
                            xq[:, kb * P:(kb + 1) * P], ident)
                    nc.vector.tensor_copy(
                        out=xT[:, g4 * 4:(g4 + 1) * 4, :],
                        in_=pt.rearrange("p (j t) -> p j t", j=4))

            # --- 1: L1 matmuls + relu^2 + stats + spill ---
            for hc in range(HC):
                w1c = w1p.tile([P, KB1, HN], bf16)
                nc.sync.dma_start(out=w1c, in_=w1_ext[hc])
                for tt in range(TT):
                    ps = ps1p.tile([P, HN], f32)
                    for kb in range(KB1):
                        nc.tensor.matmul(
                            ps, lhsT=xTs[tt][:, kb, :],
                            rhs=w1c[:, kb, :],
                            start=(kb == 0), stop=(kb == KB1 - 1))
                    z1 = z1p.tile([P, HN], f32)
                    nc.scalar.activation(out=z1, in_=ps, func=AF.Relu)
                    zq = zqp.tile([P, HN], f32)
                    nc.scalar.activation(out=zq, in_=z1, func=AF.Square)
                    nc.vector.tensor_reduce(
                        out=gxs[tt][:, hc:hc + 1], in_=zq, axis=AX.X, op=OP.max)
                    zsq = zsqp.tile([P, HN], f32)
                    nc.scalar.activation(out=zsq, in_=zq, func=AF.Square,
                                         accum_out=sqs[tt][:, hc:hc + 1])
                    nc.gpsimd.dma_start(out=z_drams[tt][hc], in_=zq)

            # ---------- phase 1.5: per-token-tile layer-2 scales ----------
            # Emitted per tt so token tile 0's layer-2 pipeline can start
            # while layer 1 is still finishing later token tiles.
            for tt in range(TT):
                nc.vector.tensor_reduce(out=col(tt, C_GMX), in_=gxs[tt],
                                        axis=AX.X, op=OP.max)
                # E[g_int^2] = sum(z^4)/H (direct sum: no variance cancellation)
                nc.vector.tensor_reduce(out=col(tt, C_MSQ), in_=sqs[tt],
                                        axis=AX.X, op=OP.add)
                nc.vector.tensor_scalar_mul(out=col(tt, C_MSQ),
                                            in0=col(tt, C_MSQ), scalar1=1.0 / H)
                nc.vector.tensor_tensor(out=col(tt, C_A4), in0=col(tt, C_A2),
                                        in1=col(tt, C_A2), op=OP.mult)
                nc.vector.tensor_tensor(out=col(tt, C_MSQ), in0=col(tt, C_MSQ),
                                        in1=col(tt, C_A4), op=OP.mult)
                # rsd = rsqrt(E + eps), Newton-refined (reuse VA/N1/N2 cols)
                nc.vector.tensor_scalar(out=col(tt, C_VA), in0=col(tt, C_MSQ),
                                        scalar1=EPS, scalar2=None, op0=OP.add)
                nc.scalar.activation(out=col(tt, C_MSQ), in_=col(tt, C_VA),
                                     func=AF.Sqrt)
                nc.vector.reciprocal(out=col(tt, C_MSQ), in_=col(tt, C_MSQ))
                nc.vector.tensor_tensor(out=col(tt, C_N1), in0=col(tt, C_MSQ),
                                        in1=col(tt, C_MSQ), op=OP.mult)
                nc.vector.tensor_tensor(out=col(tt, C_N1), in0=col(tt, C_N1),
                                        in1=col(tt, C_VA), op=OP.mult)
                nc.vector.tensor_scalar(out=col(tt, C_N1), in0=col(tt, C_N1),
                                        scalar1=3.0, scalar2=-0.5,
                                        op0=OP.subtract, op1=OP.mult)
                nc.vector.tensor_tensor(out=col(tt, C_MSQ), in0=col(tt, C_MSQ),
                                        in1=col(tt, C_N1), op=OP.mult)
                # max|g_n| = gmax_int * alpha^2 * rsd, clipped
                nc.vector.tensor_tensor(out=col(tt, C_TB), in0=col(tt, C_GMX),
                                        in1=col(tt, C_A2), op=OP.mult)
                nc.vector.tensor_tensor(out=col(tt, C_TB), in0=col(tt, C_TB),
                                        in1=col(tt, C_MSQ), op=OP.mult)
                nc.vector.tensor_scalar_max(out=col(tt, C_TB), in0=col(tt, C_TB),
                                            scalar1=EPS)
                # beta = maxgn_c * meanabsW2/127
                nc.vector.tensor_scalar(out=col(tt, C_BETA), in0=col(tt, C_TB),
                                        scalar1=cst[:, 1:2], scalar2=None,
                                        op0=OP.mult)
                # sgam = alpha^2 * rsd * 127 / maxgn_c (Newton-refined recip)
                nc.vector.reciprocal(out=col(tt, C_N1), in_=col(tt, C_TB))
                nc.vector.tensor_tensor(out=col(tt, C_N2), in0=col(tt, C_N1),
                                        in1=col(tt, C_TB), op=OP.mult)
                nc.vector.tensor_scalar(out=col(tt, C_N2), in0=col(tt, C_N2),
                                        scalar1=2.0, scalar2=-1.0,
                                        op0=OP.subtract, op1=OP.mult)
                nc.vector.tensor_tensor(out=col(tt, C_N1), in0=col(tt, C_N1),
                                        in1=col(tt, C_N2), op=OP.mult)
                nc.vector.tensor_tensor(out=col(tt, C_SGAM), in0=col(tt, C_A2),
                                        in1=col(tt, C_MSQ), op=OP.mult)
                nc.vector.tensor_tensor(out=col(tt, C_SGAM), in0=col(tt, C_SGAM),
                                        in1=col(tt, C_N1), op=OP.mult)
                nc.vector.tensor_scalar_mul(out=col(tt, C_SGAM),
                                            in0=col(tt, C_SGAM), scalar1=127.0)

        # ---------------- phase 2: layer 2 ----------------
        with tc.tile_pool(name="zl", bufs=3) as zlp, \
             tc.tile_pool(name="t2", bufs=2) as t2p, \
             tc.tile_pool(name="gq", bufs=2) as gqp, \
             tc.tile_pool(name="gst", bufs=3) as gstp, \
             tc.tile_pool(name="gTt", bufs=3) as gTp, \
             tc.tile_pool(name="w2", bufs=2) as w2p, \
             tc.tile_pool(name="ysb", bufs=2) as yp, \
             tc.tile_pool(name="ps2", bufs=3, space="PSUM") as ps2p, \
             tc.tile_pool(name="psT2", bufs=2, space="PSUM") as psT2p:

            # --- 2a: reload integer g, quantize, PE-transpose, spill to DRAM
            # in a partition-contiguous [P, KB2, P] layout per token tile ---
            for tt in range(TT):
                for hc in range(HC):
                    zl = zlp.tile([P, HN], f32)
                    nc.scalar.dma_start(out=zl, in_=z_drams[tt][hc])
                    t2 = t2p.tile([P, HN], f32)
                    nc.vector.tensor_scalar(out=t2, in0=zl,
                                            scalar1=scs[tt][:, 16:17],
                                            scalar2=MAGIC, op0=OP.mult, op1=OP.add)
                    gq = gqp.tile([P, HN], bf16)
                    nc.scalar.activation(out=gq, in_=t2, func=AF.Identity,
                                         bias=nmag_c)
                    pt = psT2p.tile([P, 4 * P], bf16)
                    for j in range(4):
                        nc.tensor.transpose(
                            pt[:, j * P:(j + 1) * P],
                            gq[:, j * P:(j + 1) * P], ident)
                    gst = gstp.tile([P, 4, P], bf16)
                    nc.vector.tensor_copy(
                        out=gst, in_=pt.rearrange("p (j t) -> p j t", j=4))
                    nc.gpsimd.dma_start(
                        out=gqT_drams[tt][:, hc * 4:(hc + 1) * 4, :], in_=gst)

            # --- 2b: L2 matmuls, N=512 streams; gT tiles stream back via
            # plain contiguous DMA (16 KB per partition line) ---
            for dc in range(DC):
                w2c = w2p.tile([P, KB2, DN], bf16)
                nc.scalar.dma_start(out=w2c, in_=w2_ext[dc])
                for tt in range(TT):
                    gTt = gTp.tile([P, KB2, P], bf16)
                    nc.sync.dma_start(out=gTt, in_=gqT_drams[tt][:])
                    ps = ps2p.tile([P, DN], f32)
                    for kb in range(KB2):
                        nc.tensor.matmul(
                            ps, lhsT=gTt[:, kb, :],
                            rhs=w2c[:, kb, :],
                            start=(kb == 0), stop=(kb == KB2 - 1))
                    ysb = yp.tile([P, DN], f32)
                    nc.scalar.activation(out=ysb, in_=ps, func=AF.Copy,
                                         scale=scs[tt][:, 15:16])
                    nc.scalar.dma_start(
                        out=y_ext[tt * P:(tt + 1) * P, dc * DN:(dc + 1) * DN],
                        in_=ysb)

    nc.compile()
    return nc


def _host_prep(W1, W2):
    """Quantize weights exactly as the jax fp32 reference does (on CPU)."""
    import jax
    import jax.numpy as jnp

    cpu = jax.devices("cpu")[0]
    out = {}
    for name, W in (("w1", W1), ("w2", W2)):
        with jax.default_device(cpu):
            Wj = jnp.asarray(W)
            scale = 1.0 / jnp.clip(jnp.mean(jnp.abs(Wj)), EPS)
            Wq = jnp.clip(jnp.round(Wj * scale), -1.0, 1.0)
            out[name + "_int"] = np.asarray(Wq).astype(np.float32)
            # effective per-tensor weight magnitude, as the fp32 reference divides
            out[name + "_q"] = float(np.float32(1.0) / np.float32(scale))
    return out


def prep_weight_inputs(W1, W2):
    """Common per-core input tensors (weights + constants)."""
    wp = _host_prep(np.asarray(W1, np.float32), np.asarray(W2, np.float32))
    bf16 = ml_dtypes.bfloat16
    # W1int [H, D] -> W1^T [D, H] -> [hc, p, kb, hn]
    w1t = np.ascontiguousarray(
        wp["w1_int"].T.reshape(KB1, P, HC, HN).transpose(2, 1, 0, 3)
    ).astype(bf16)
    # W2int [D, H] -> W2^T [H, D] -> [dc, p, kb, dn]
    w2t = np.ascontiguousarray(
        wp["w2_int"].T.reshape(KB2, P, DC, DN).transpose(2, 1, 0, 3)
    ).astype(bf16)
    cst = np.zeros((1, 8), np.float32)
    cst[0, 0] = np.float32(wp["w1_q"]) / np.float32(127.0)
    cst[0, 1] = np.float32(wp["w2_q"]) / np.float32(127.0)
    return {"w1t": w1t, "w2t": w2t, "cst": cst}


def kernel(x, W1, W2):
    from concourse.bass_utils import run_bass_kernel_spmd

    if "nc" not in _CACHE:
        _CACHE["nc"] = _build_nc()
    nc = _CACHE["nc"]

    xf = np.ascontiguousarray(x.reshape(-1, D).astype(np.float32, copy=False))
    common = prep_weight_inputs(W1, W2)
    in_maps = [{"x": xf[c * TOK:(c + 1) * TOK], **common} for c in range(N_CORES)]

    res = run_bass_kernel_spmd(nc, in_maps, list(range(N_CORES)),
                               trace=_CACHE.get("trace", False))
    _CACHE["last_res"] = res
    y = np.concatenate([res.results[c]["y"] for c in range(N_CORES)], axis=0)
    return y.reshape(x.shape[0], x.shape[1], D).astype(np.float32, copy=False)
